# revision 22
# baseline (speedup 1.0000x reference)
"""Trainium2 kernel for: out = (mat1 @ mat2 + input_tensor).astype(f32), all int32 in [0,16).

Strategy
--------
Values are integers in [0, 15], so:
  - mat1/mat2 are exact in fp8 e4m3 (integers 0..15 need 4 significand bits; e4m3 has 4)
  - products (<= 225) are exact in the PE datapath (e6m3 upcast -> e10m10 product)
  - accumulators (<= 15*15*4096 + 15 = 921,615 < 2^24) are exact in fp32 PSUM
so an fp8 DoubleRow matmul (2 MACs/cell/cycle, the fastest PE mode on trn2)
reproduces the int32 reference bit-exactly in fp32.

Sharding: 2D, 4 mat1-row blocks x 2 mat2-column blocks over 8 cores. Each
core computes a [1024, 2048] slab of the output. Pure SPMD, no collectives.
(4x2 minimizes per-core DMA: 4 MiB mat1 + 8 MiB mat2 + 4 MiB input + 8 MiB
out = 24 MiB vs 30 MiB for 8x1 with replicated mat2.)

Per-core device program:
  - mat1 block resident in SBUF (fp8, DoubleRow-interleaved on host), loaded
    as one tile per 128-row m-tile so the first matmuls start early
  - mat2 streamed one n-block (512 cols, all of K) per DMA: a single
    [128, 16 KiB] contiguous transfer, double-buffered; each is 16 DoubleRow
    k-tiles
  - 16 DoubleRow matmuls (K=256 each) accumulate a [128, 512] fp32 PSUM bank;
    8 m-tiles use all 8 PSUM banks per n-block
  - DVE adds the (bf16) input_tensor tile during PSUM->SBUF eviction
  - one batched fp32 output DMA per n-block
All DMAs are whole-partition-line contiguous, and there are only ~29 DMA
instructions total (DMA-instruction issue on the Sync engine costs ~0.6 us
each, which was the v1 bottleneck).
"""

import numpy as np
import ml_dtypes

import concourse.bass as bass
import concourse.mybir as mybir
import concourse.tile as tile
from concourse import bacc
from concourse.bass import ts
from concourse.bass_utils import run_bass_kernel_spmd

F8 = mybir.dt.float8e4
BF16 = mybir.dt.bfloat16
F32 = mybir.dt.float32

N_CORES = 8
A_SHARD = 4  # mat1 row blocks
B_SHARD = 2  # mat2 col blocks
P = 128  # partitions
NB_TILE = 512  # output free-dim tile (one PSUM bank of fp32)
KP = 256  # contraction per DoubleRow matmul (2 x 128)


def build_program(m_shard: int, K: int, n_shard: int) -> bass.Bass:
    """One NeuronCore's program: [m_shard, K] @ [K, n_shard] + input -> fp32.

    DRAM parameter layouts (host pre-packs; p is the SBUF partition index):
      m1  [MT, P, KT, 2, P] fp8    : m1[mt, p, kt, i, m] = mat1_blk[P*mt + m, KP*kt + 128*i + p]
      m2  [NB, P, KT, 2, 512] fp8  : m2[nb, p, kt, i, n] = mat2_blk[KP*kt + 128*i + p, 512*nb + n]
      inp [NB, P, MT, 512] bf16    : inp[nb, p, mt, n] = input_blk[P*mt + p, 512*nb + n]
      out [NB, P, MT, 512] f32     : out[nb, p, mt, n] = result[P*mt + p, 512*nb + n]
    """
    KT = K // KP
    MT = m_shard // P
    NB = n_shard // NB_TILE

    nc = bacc.Bacc("TRN2", target_bir_lowering=False, debug=False)
    m1d = nc.dram_tensor("m1", [MT, P, KT, 2, P], F8, kind="ExternalInput")
    m2d = nc.dram_tensor("m2", [NB, P, KT, 2, NB_TILE], F8, kind="ExternalInput")
    inpd = nc.dram_tensor("inp", [NB, P, MT, NB_TILE], F8, kind="ExternalInput")
    outd = nc.dram_tensor("out", [NB, P, MT, NB_TILE], F32, kind="ExternalOutput")

    with tile.TileContext(nc) as tc:
        with (
            tc.tile_pool(name="m1", bufs=MT) as m1_pool,
            tc.tile_pool(name="m2", bufs=2) as m2_pool,
            tc.tile_pool(name="inp", bufs=2) as inp_pool,
            tc.tile_pool(name="res", bufs=2) as res_pool,
            tc.tile_pool(name="psum", bufs=8, space="PSUM") as psum_pool,
        ):
            # First m-tile of weights first, so matmuls can start ASAP.
            m1_tiles = [
                m1_pool.tile([P, KT, 2, P], F8, name="m1_0", tag="m1_0", bufs=1)
            ]
            nc.sync.dma_start(m1_tiles[0][:], m1d[0])

            # PE warmup: ~45 throwaway matmuls on a zeroed tile while the
            # input DMAs stream in. The PE clock sits at 1.2 GHz until the
            # HAM sees ~3.4us of sustained activity; burning that window
            # during the DMA head means the real matmuls start at 2.4 GHz.
            warm_src = inp_pool.tile([P, P], F8, tag="warm", bufs=1)
            nc.gpsimd.memset(warm_src[:], 0.0)
            # Shares the "ps" tag (and its 8 PSUM-bank slots) with the real
            # accumulators; released before the first real one is needed.
            warm_ps = psum_pool.tile([P, NB_TILE], F32, tag="ps")
            for _ in range(50):
                nc.tensor.matmul(
                    warm_ps[:, :P], warm_src[:], warm_src[:], start=True, stop=True
                )

            def load_nb(nb, split=1, inp_too=True):
                m2s = m2_pool.tile([P, KT, 2, NB_TILE], F8, tag="m2")
                step = KT // split
                for h in range(split):
                    ks = slice(h * step, (h + 1) * step)
                    nc.sync.dma_start(m2s[:, ks], m2d[nb, :, ks])
                inps = inp_pool.tile([P, MT, NB_TILE], F8, tag="inp")
                if inp_too:
                    nc.sync.dma_start(inps[:], inpd[nb])
                return m2s, inps

            # DMA-queue order tracks first-use order: the first half of
            # nb=0's m2 (in quarters, so per-kt deps release matmuls as each
            # chunk lands), then the remaining weight tiles (consumed at
            # ~1.8us per tile by the phase-a sweep), then the second half.
            m2s0 = m2_pool.tile([P, KT, 2, NB_TILE], F8, tag="m2", name="m2s0")
            chunk = max(1, KT // 4)
            for k0 in range(0, KT // 2, chunk):
                ks = slice(k0, min(k0 + chunk, KT // 2))
                nc.sync.dma_start(m2s0[:, ks], m2d[0, :, ks])
            for mt in range(1, MT):
                t = m1_pool.tile([P, KT, 2, P], F8, tag=f"m1_{mt}", bufs=1)
                nc.sync.dma_start(t[:], m1d[mt])
                m1_tiles.append(t)
            for k0 in range(KT // 2, KT, chunk):
                ks = slice(k0, min(k0 + chunk, KT))
                nc.sync.dma_start(m2s0[:, ks], m2d[0, :, ks])
            inps0 = inp_pool.tile([P, MT, NB_TILE], F8, tag="inp", name="inps0")
            nc.sync.dma_start(inps0[:], inpd[0])
            cur = (m2s0, inps0)

            for nb in range(NB):
                m2s, inps = cur
                if nb + 1 < NB:
                    # Prefetch before this nb's stores hit the Sync queue
                    # (a waiting store would head-of-line-block the queue).
                    cur = load_nb(nb + 1)
                outs = res_pool.tile([P, MT, NB_TILE], F32)
                # nb=0 runs its contraction in two kt-phases across all
                # m-tiles: compute on the first half of m2 starts while the
                # second half is still in flight, so the PE never outruns
                # the initial DMA ramp into a >3.4us (HAM-rethrottling)
                # stall. Later nbs have fully-prefetched m2: one phase.
                phases = [(0, KT // 2), (KT // 2, KT)] if nb == 0 else [(0, KT)]
                pss = [psum_pool.tile([P, NB_TILE], F32, name=f"ps_{mt}", tag="ps")
                       for mt in range(MT)]
                for k0, k1 in phases:
                    for mt in range(MT):
                        for kt in range(k0, k1):
                            nc.tensor.matmul(
                                pss[mt][:],
                                m1_tiles[mt][:, kt],
                                m2s[:, kt],
                                start=(kt == 0),
                                stop=(kt == KT - 1),
                                perf_mode=mybir.MatmulPerfMode.DoubleRow,
                            )
                for mt in range(MT):
                    nc.vector.tensor_add(outs[:, mt], pss[mt][:], inps[:, mt])
                    # Per-mt store: drains each result as soon as its add
                    # lands; keeps the kernel tail to one tile, not one nb.
                    nc.sync.dma_start(outd[nb, :, mt], outs[:, mt])
    nc.compile()
    return nc


def pack_m1_block(blk: np.ndarray) -> np.ndarray:
    """[m_shard, K] int -> [MT, P, KT, 2, P] fp8 (DoubleRow weight layout)."""
    m_shard, K = blk.shape
    # [mt, m, kt, i, p] from blk[P*mt + m, KP*kt + 128*i + p]
    r = blk.reshape(m_shard // P, P, K // KP, 2, P)
    return np.ascontiguousarray(r.transpose(0, 4, 2, 3, 1)).astype(np.float32).astype(
        ml_dtypes.float8_e4m3
    )


def pack_m2(mat2: np.ndarray) -> np.ndarray:
    """[K, N] int -> [N//512, P, KT, 2, 512] fp8 (DoubleRow moving layout)."""
    K, N = mat2.shape
    r = mat2.reshape(K // KP, 2, P, N // NB_TILE, NB_TILE)  # [kt, i, p, nb, n]
    return np.ascontiguousarray(r.transpose(3, 2, 0, 1, 4)).astype(np.float32).astype(
        ml_dtypes.float8_e4m3
    )


def pack_inp_block(blk: np.ndarray) -> np.ndarray:
    """[m_shard, n_shard] int -> [NB, P, MT, 512] fp8 (0..15 are exact)."""
    m_shard, n_shard = blk.shape
    r = blk.reshape(m_shard // P, P, n_shard // NB_TILE, NB_TILE)  # [mt, p, nb, n]
    return (
        np.ascontiguousarray(r.transpose(2, 1, 0, 3))
        .astype(np.float32)
        .astype(ml_dtypes.float8_e4m3)
    )


def unpack_out(packed: np.ndarray, m_shard: int, n_shard: int) -> np.ndarray:
    """[NB, P, MT, 512] f32 -> [m_shard, n_shard] f32."""
    return np.ascontiguousarray(packed.transpose(2, 1, 0, 3)).reshape(m_shard, n_shard)


def _prepare(input_tensor, mat1, mat2):
    input_tensor = np.asarray(input_tensor)
    mat1 = np.asarray(mat1)
    mat2 = np.asarray(mat2)
    M, K = mat1.shape
    N = mat2.shape[1]
    m_shard = M // A_SHARD
    n_shard = N // B_SHARD
    nb_per_core = n_shard // NB_TILE

    nc = build_program(m_shard, K, n_shard)

    m2p = pack_m2(mat2)  # [N//512, P, KT, 2, 512]; core takes its nb range
    in_maps = []
    for c in range(N_CORES):
        ra, cb = divmod(c, B_SHARD)
        rows = slice(ra * m_shard, (ra + 1) * m_shard)
        cols = slice(cb * n_shard, (cb + 1) * n_shard)
        nbs = slice(cb * nb_per_core, (cb + 1) * nb_per_core)
        in_maps.append(
            {
                "m1": pack_m1_block(mat1[rows]),
                "m2": m2p[nbs],
                "inp": pack_inp_block(input_tensor[rows, cols]),
            }
        )
    return nc, in_maps, (m_shard, n_shard)


def _gather(results, m_shard, n_shard):
    M = m_shard * A_SHARD
    N = n_shard * B_SHARD
    out = np.empty((M, N), dtype=np.float32)
    for c in range(N_CORES):
        ra, cb = divmod(c, B_SHARD)
        out[
            ra * m_shard : (ra + 1) * m_shard, cb * n_shard : (cb + 1) * n_shard
        ] = unpack_out(results[c]["out"], m_shard, n_shard)
    return out


def kernel(input_tensor, mat1, mat2):
    nc, in_maps, (m_shard, n_shard) = _prepare(input_tensor, mat1, mat2)
    res = run_bass_kernel_spmd(nc, in_maps, list(range(N_CORES))).results
    return _gather(res, m_shard, n_shard)


def kernel_traced(input_tensor, mat1, mat2, **kwargs):
    """Like kernel(), but also returns BassKernelResults (exec_time_ns etc.)."""
    nc, in_maps, (m_shard, n_shard) = _prepare(input_tensor, mat1, mat2)
    res = run_bass_kernel_spmd(
        nc, in_maps, list(range(N_CORES)), trace=True, **kwargs
    )
    return _gather(res.results, m_shard, n_shard), res
